# revision 19
# baseline (speedup 1.0000x reference)
"""MoCo hard-example-mining loss (topk_masking) on 8 Trainium2 NeuronCores.

Strategy (sharding_hint: shard queue along K):
  The reference computes dist = euclid(feat_q, queue_eff.T) [N=512, K=65536],
  then masked max (hard positive) / min (hard negative) per row, then a
  scalar soft-margin loss.  After the enqueue step, queue_eff columns are:
    - cols [0, 512):  feat_k.T with labels = targets   (the "special" block)
    - cols [512, 64K): original L2-normalized queue columns, labels = 0
  For the zero-label region the mask is row-constant and ||y_j||^2 == 1, so
  per row only ONE of max_j / min_j of p_ij = <feat_q_i, y_j> is needed:
    - rows with target == 0 use the region as positives -> need max dist
      -> need MIN_j p_ij
    - rows with target != 0 use the region as negatives -> need min dist
      -> need MAX_j p_ij
  Flipping the sign of feat_q rows with target != 0 on the host turns both
  cases into a single MIN over j, halving the on-device reduction work.

  Device (per core, queue sharded along K, 8192 cols each):
    P = feat_q' @ slab in fp8 (DoubleRow perf mode: 256-deep contraction
    per matmul, 2x bf16 throughput, half the HBM traffic), then a PSUM
    drain computing the per-row running MIN, split between the Vector
    engine (direct PSUM tensor_reduce) and the Scalar engine (copy to
    bf16 SBUF) + Vector (bf16 tensor-tensor min at 2x) so both engines
    share the PSUM read bandwidth.  Output: [128, 4] per-row minima.
  The 512-column special block and the final scalar loss are computed
  exactly on the host in float64 (trivial cost).
"""

import sys
import types
import numpy as np
import ml_dtypes

N, DIM, K, B = 512, 512, 65536, 512
NCORES = 8
KZ = K - B            # zero-label columns (65024)
CPC = K // NCORES     # padded columns per core (8192)
NT = CPC // 512       # 512-wide column tiles per core (16)
BIG = 9999999.0
SCALE_Q = 16.0        # fp8 scaling for feat_q rows
SCALE_Z = 64.0        # fp8 scaling for queue columns (entries ~N(0, 1/512))
PSCALE = SCALE_Q * SCALE_Z

# Drain-path assignment per 2-bank group g = n*2 + h (n tile, h m-pair):
#   'D' = DVE tensor_reduce directly from PSUM
#   'A' = ACT copy (PSUM fp32 -> SBUF bf16) + DVE tensor-tensor min (2x)
#         into a per-phase running tile
# Interleaved so Vector and Scalar stream concurrently (the Pool engine has
# no min/max ALU support so GpSimd cannot help, and the custom
# tensor_tensor_reduce DVE op wedges this runtime).
# g29/g30 drain on DVE directly and g31 via ACT+DVE-bf16-reduce so the two
# engines work the last banks in parallel; phase 2 closes at g28 so its
# fold overlaps the final groups.
D_SET = {2, 6, 10, 14, 18, 22, 29, 30}
PATHS = ["D" if g in D_SET else ("S" if g == 31 else "A") for g in range(32)]

LAST_RESULTS = None   # BassKernelResults of the most recent device run
_NC_CACHE = {}


def _install_axon_hooks_shim():
    """antenv.axon_hooks is absent on this image; bass_utils imports it when
    NTFF tracing is requested.  Provide the tiny get/set module and register
    the ctypes-based NTFF hook so trace=True / BASS_TRACE=1 works."""
    try:
        import antenv  # noqa: F401
    except ImportError:
        return
    if "antenv.axon_hooks" in sys.modules:
        return
    mod = types.ModuleType("antenv.axon_hooks")
    mod._hook = None

    def set_axon_ntff_profile_hook(h):
        mod._hook = h

    def get_axon_ntff_profile_hook():
        return mod._hook

    mod.set_axon_ntff_profile_hook = set_axon_ntff_profile_hook
    mod.get_axon_ntff_profile_hook = get_axon_ntff_profile_hook
    sys.modules["antenv.axon_hooks"] = mod
    sys.modules["antenv"].axon_hooks = mod
    try:
        from trn_agent_boot.trn_boot import _ntff_profile_via_ctypes

        mod._hook = _ntff_profile_via_ctypes("/opt/axon/libaxon_pjrt.so")
    except Exception:
        pass


def _build_nc():
    """Build + compile the per-core Bass program (identical on all cores)."""
    import concourse.bacc as bacc
    import concourse.mybir as mybir
    from concourse.tile import TileContext

    bf16 = mybir.dt.bfloat16
    f32 = mybir.dt.float32
    f8 = mybir.dt.float8e4
    DR = mybir.MatmulPerfMode.DoubleRow
    MIN = mybir.AluOpType.min
    AX = mybir.AxisListType.X

    nc = bacc.Bacc("TRN2", debug=False, target_bir_lowering=False)
    # fp8 layouts keyed for DoubleRow: logical contraction index
    # d = kk*256 + i*128 + p  (kk = matmul chunk, i = DoubleRow pair, p = part)
    qT = nc.dram_tensor("qT", [128, 2 * 2 * N], f8, kind="ExternalInput")
    slab = nc.dram_tensor("slab", [128, NT * 2 * 2 * 512], f8, kind="ExternalInput")
    # o[:, m] = min over this core's columns of P'[m*128 + p, :]
    o = nc.dram_tensor("o", [128, 4], f32, kind="ExternalOutput")

    qT_v = qT.ap().rearrange("p (k i m) -> p k i m", k=2, i=2)
    slab_v = slab.ap().rearrange("p (n k i c) -> p n k i c", n=NT, k=2, i=2)

    with TileContext(nc) as tc:
        with (
            tc.tile_pool(name="qpool", bufs=1) as qpool,
            tc.tile_pool(name="spool", bufs=10) as spool,
            tc.tile_pool(name="bpool", bufs=8) as bpool,
            tc.tile_pool(name="rpool", bufs=1) as rpool,
            tc.tile_pool(name="opool", bufs=1) as opool,
            tc.tile_pool(name="pspool", bufs=4, space="PSUM") as pspool,
        ):
            # HAM warmup: tiny matmuls during the preamble/DMA fill so the PE
            # clock gate is at 8/8 when the real stream starts; sized to
            # bridge until the first slab tile lands (~3.2us)
            warm = qpool.tile([128, 16], bf16, name="warm")
            nc.gpsimd.memset(warm, 0.0)
            wps = pspool.tile([128, 2, 512], f32, name="wps", tag="ps2")
            for _ in range(80):
                nc.tensor.matmul(wps[0:16, 0, 0:16], warm, warm)

            # A-path running-min tiles (two phases so the phase-1 reduce
            # overlaps the phase-2 stream), D-path output columns
            rmn1 = rpool.tile([128, 4, 512], bf16, name="rmn1")
            rmn2 = rpool.tile([128, 4, 512], bf16, name="rmn2")
            osbD = opool.tile([128, 4, 8], f32, name="osbD")
            oA1 = opool.tile([128, 4], f32, name="oA1")
            oA2 = opool.tile([128, 4], f32, name="oA2")
            ofin = opool.tile([128, 4], f32, name="ofin")
            nc.gpsimd.memset(osbD, BIG)

            # stage qT + the first slab tile as two whole-tile DMAs (splitting
            # them into chunks costs more in per-DMA latency than it saves)
            qt = qpool.tile([128, 2, 2, N], f8, name="qt")
            nc.sync.dma_start(out=qt, in_=qT_v)
            st0 = spool.tile([128, 2, 2, 512], f8, name="st", tag="st")
            nc.sync.dma_start(out=st0, in_=slab_v[:, 0])

            dcol = [0, 0]          # next osbD column per m-pair
            seen = {}              # (phase, h) -> running tile initialized

            def fold_reduce(rmn, oA):
                # min-fold 512 -> 128 with cheap 2x tensor-tensor ops, then
                # one short 1x tensor_reduce (beats a full-width reduce)
                nc.vector.tensor_tensor(
                    rmn[:, :, 0:256], rmn[:, :, 0:256], rmn[:, :, 256:512],
                    op=MIN,
                )
                nc.vector.tensor_tensor(
                    rmn[:, :, 0:128], rmn[:, :, 0:128], rmn[:, :, 128:256],
                    op=MIN,
                )
                nc.vector.tensor_reduce(oA, rmn[:, :, 0:128], axis=AX, op=MIN)
            for n in range(NT):
                if n == 0:
                    st = st0
                else:
                    st = spool.tile([128, 2, 2, 512], f8, name="st", tag="st")
                    nc.sync.dma_start(out=st, in_=slab_v[:, n])
                for h in range(2):
                    g = n * 2 + h
                    ps = pspool.tile([128, 2, 512], f32, name="ps", tag="ps2")
                    for mloc in range(2):
                        m = 2 * h + mloc
                        for kk in range(2):
                            nc.tensor.matmul(
                                ps[:, mloc, :],
                                qt[:, kk, :, m * 128 : (m + 1) * 128],
                                st[:, kk],
                                start=(kk == 0),
                                stop=(kk == 1),
                                perf_mode=DR,
                            )
                    if PATHS[g] == "D":
                        j = dcol[h]
                        dcol[h] += 1
                        nc.vector.tensor_reduce(
                            osbD[:, 2 * h : 2 * h + 2, j : j + 1], ps,
                            axis=AX, op=MIN,
                        )
                    elif PATHS[g] == "S":
                        bt = bpool.tile([128, 2, 512], bf16, name="bt", tag="bt")
                        nc.scalar.copy(bt, ps)
                        j = dcol[h]
                        dcol[h] += 1
                        nc.vector.tensor_reduce(
                            osbD[:, 2 * h : 2 * h + 2, j : j + 1], bt,
                            axis=AX, op=MIN,
                        )
                    else:
                        bt = bpool.tile([128, 2, 512], bf16, name="bt", tag="bt")
                        nc.scalar.copy(bt, ps)
                        phase = 0 if g < 16 else 1
                        rmn = rmn1 if phase == 0 else rmn2
                        sl = rmn[:, 2 * h : 2 * h + 2, :]
                        if (phase, h) in seen:
                            nc.vector.tensor_tensor(sl, sl, bt, op=MIN)
                        else:
                            nc.vector.tensor_copy(sl, bt)
                            seen[(phase, h)] = True
                    if g == 15:
                        fold_reduce(rmn1, oA1)
                    if g == 28:
                        fold_reduce(rmn2, oA2)

            nc.vector.tensor_reduce(ofin, osbD, axis=AX, op=MIN)
            nc.vector.tensor_tensor(ofin, ofin, oA1, op=MIN)
            nc.vector.tensor_tensor(ofin, ofin, oA2, op=MIN)
            nc.sync.dma_start(out=o.ap(), in_=ofin)

    nc.compile()
    return nc


def _get_nc():
    if "nc" not in _NC_CACHE:
        _install_axon_hooks_shim()
        _NC_CACHE["nc"] = _build_nc()
    return _NC_CACHE["nc"]


def _host_reference(feat_q, feat_k, targets, queue, queue_label):
    """Exact numpy fallback (float64) — used only if input assumptions
    (zero labels / normalized columns outside the enqueue block) fail."""
    fq = feat_q.astype(np.float64)
    fk = feat_k.astype(np.float64)
    t = targets.astype(np.int64)
    q = queue.astype(np.float64).copy()
    ql = queue_label.astype(np.int64).copy()
    q[:, : fk.shape[0]] = fk.T
    ql[: fk.shape[0]] = t
    xx = (fq * fq).sum(1)[:, None]
    yy = (q * q).sum(0)[None, :]
    sq = xx + yy - 2.0 * (fq @ q)
    dist = np.sqrt(np.clip(sq, 1e-12, None))
    is_pos = t[:, None] == ql[None, :]
    dist_ap = np.max(dist - BIG * (~is_pos), axis=1)
    dist_an = np.min(dist + BIG * is_pos, axis=1)
    return _loss(dist_ap, dist_an)


def _loss(dist_ap, dist_an):
    diff = dist_an - dist_ap
    loss_soft = np.mean(np.logaddexp(0.0, -diff))
    if np.isinf(loss_soft):
        return np.float32(np.mean(np.maximum(dist_ap - dist_an + 0.3, 0.0)))
    return np.float32(loss_soft)


def _to_fp8(x):
    return np.clip(x, -240.0, 240.0).astype(ml_dtypes.float8_e4m3fn)


def kernel(feat_q, feat_k, targets, queue, queue_label):
    feat_q = np.asarray(feat_q, dtype=np.float32)
    feat_k = np.asarray(feat_k, dtype=np.float32)
    targets = np.asarray(targets)
    queue = np.asarray(queue, dtype=np.float32)
    queue_label = np.asarray(queue_label)

    t = targets.astype(np.int64)
    Z = queue[:, B:]  # zero-label region, untouched by the enqueue

    # Guards for the two structural assumptions this split relies on.
    ok = not np.any(queue_label != 0)
    if ok:
        sample = np.linspace(0, KZ - 1, 512, dtype=np.int64)
        yy_s = np.einsum("ij,ij->j", Z[:, sample], Z[:, sample], dtype=np.float64)
        ok = bool(np.max(np.abs(yy_s - 1.0)) < 1e-3)
    if not ok:
        return _host_reference(feat_q, feat_k, targets, queue, queue_label)

    # ---- device part: per-row min of feat_q' @ Z over the zero-label region
    # (rows with target != 0 are sign-flipped so their max becomes a min)
    s = np.where(t != 0, -1.0, 1.0).astype(np.float32)
    Qs = feat_q * (s * SCALE_Q)[:, None]
    # qT_dr[p, kk, i, m] = Qs[m, kk*256 + i*128 + p]
    qT_dr = np.ascontiguousarray(
        _to_fp8(Qs.T).reshape(2, 2, 128, N).transpose(2, 0, 1, 3).reshape(128, -1)
    )
    Z8 = _to_fp8(Z * SCALE_Z)
    in_maps = []
    for c in range(NCORES):
        lo = c * (KZ // NCORES)
        hi = lo + (KZ // NCORES)
        sl = np.empty((DIM, CPC), dtype=ml_dtypes.float8_e4m3fn)
        sl[:, : hi - lo] = Z8[:, lo:hi]
        if hi - lo < CPC:  # pad the tail with duplicate columns
            sl[:, hi - lo :] = Z8[:, : CPC - (hi - lo)]
        # slab_dr[p, n, kk, i, c] = sl[kk*256 + i*128 + p, n*512 + c]
        sl_dr = np.ascontiguousarray(
            sl.reshape(2, 2, 128, NT, 512).transpose(2, 3, 0, 1, 4).reshape(128, -1)
        )
        in_maps.append({"qT": qT_dr, "slab": sl_dr})

    from concourse import bass_utils

    nc = _get_nc()
    res = bass_utils.run_bass_kernel_spmd(nc, in_maps, core_ids=list(range(NCORES)))
    global LAST_RESULTS
    LAST_RESULTS = res

    pm = np.full((128, 4), np.inf)
    for c in range(NCORES):
        oc = np.asarray(res.results[c]["o"], dtype=np.float64)  # [128, 4]
        pm = np.minimum(pm, oc)
    pm = pm.T.reshape(N) / PSCALE  # row r = m*128 + p

    # ---- host part: special 512-column block, exact in float64
    fq = feat_q.astype(np.float64)
    fk = feat_k.astype(np.float64)
    xx = (fq * fq).sum(1)
    kk_ = (fk * fk).sum(1)
    G = fq @ fk.T
    sqB = xx[:, None] + kk_[None, :] - 2.0 * G
    distB = np.sqrt(np.clip(sqB, 1e-12, None))
    maskB = t[:, None] == t[None, :]
    apB = np.max(distB - BIG * (~maskB), axis=1)
    anB = np.min(distB + BIG * maskB, axis=1)

    # zero-label region: rows with t==0 got min_j p (for the max distance),
    # rows with t!=0 got min_j(-p) = -max_j p (for the min distance)
    ap_z = np.where(
        t == 0, np.sqrt(np.clip(xx + 1.0 - 2.0 * pm, 1e-12, None)), -BIG
    )
    an_z = np.where(
        t != 0, np.sqrt(np.clip(xx + 1.0 + 2.0 * pm, 1e-12, None)), BIG
    )

    dist_ap = np.maximum(apB, ap_z)
    dist_an = np.minimum(anB, an_z)
    return _loss(dist_ap, dist_an)
